# revision 30
# baseline (speedup 1.0000x reference)
"""Trainium2 Bass kernel for an embedding-bag + 2-layer MLP + log_softmax model.

Model (reference semantics):
  cat = cat_embeds[cat_b_ix]                       # (B, 128)
  hvb = multihot(hvb_ix) @ hvec_embeds + hvb_top   # (B, 256)  (set semantics)
  hvf = multihot(hvf_ix) @ hvec_embeds + hvf_top   # (B, 256)
  x   = [cat | hvb | hvf | d_onehot]               # (B, 647)
  h   = relu(x @ W1.T + b1)                        # (B, 1024)
  y   = h @ W2.T + b2                              # (B, 10000)
  out = log_softmax(y, axis=1)

Sharding: data-parallel over 8 NeuronCores (256 batch rows each);
embedding tables and weights replicated.

Device kernel (per core), v2:
  - batched indirect-DMA row gathers (one instruction per 8-row bag pair;
    SWDGE cost is ~1us fixed per instruction, so batching cuts gather
    dispatch from ~37us to ~6us)
  - hvec table stored fp8e4 scaled x32 (halves gather bytes); scale folded
    into W1 columns on the host, so no on-device rescale
  - L1 matmul bf16 -> h stored fp8e4 (W2 scaled x4 on host, W1/b1 scaled
    1/4, so h is h/4: keeps W2 out of the fp8 subnormal range)
  - L2 matmul in fp8 DoubleRow perf mode: 2 fp8 weights per PE cell,
    halves both the W2 HBM traffic (10.5MB vs 20.5MB) and the PE stream
    time; W2 chunks padded 500->512 cols so the k-pair stride is 16B
    aligned
  - all 20 W2 chunks prefetched up front (fp8 makes them fit in SBUF),
    DMA streams continuously
  - fused exp+row-sum on ACT (logits are small, no max-shift needed),
    PSUM->SBUF y copies alternate DVE/GPSIMD, log-softmax subtraction
    split DVE/GPSIMD, bf16 output (upcast on host)
"""

import sys
import types

import ml_dtypes
import numpy as np

BF16NP = ml_dtypes.bfloat16
FP8NP = ml_dtypes.float8_e4m3

import concourse.bacc as bacc
import concourse.bass as bass
import concourse.mybir as mybir
import concourse.tile as tile
from concourse.bass import IndirectOffsetOnAxis
from concourse.bass_utils import run_bass_kernel_spmd

F32 = mybir.dt.float32
BF16 = mybir.dt.bfloat16
FP8 = mybir.dt.float8e4
I32 = mybir.dt.int32

N_CORES = 8
B = 2048
BL = B // N_CORES          # 256 batch rows per core
NNZ = 8
CAT_V = 10000
HVEC_V = 50000
SYN = 128
SEM = 256
HIDDEN = 1024
OUT = 10000
IN_DIM = 7 + SYN + 2 * SEM     # 647
KC1 = 6                        # L1 contraction chunks (768 = 6*128, padded)
KC2 = HIDDEN // 128            # 8
OCR = 500                      # real output cols per chunk (10000 = 20*500)
OCP = 512                      # padded chunk width (16B-aligned kc stride)
NOC = OUT // OCR               # 20
NSC = NOC // 2                 # 10 super-chunks of 2 chunks each
EMB_S = 1.0                    # embedding tables kept bf16: no rescale
W2_S = 4.0                     # W2 scaled x4, h scaled 1/4

_STATE = {}


def _build_program(has_b2):
    nc = bacc.Bacc("TRN2", target_bir_lowering=False, debug=False,
                   num_devices=N_CORES, dynamic_dma_scratch_size=16384)

    idx_hv = nc.dram_tensor("idx_hv", [128, 32], I32, kind="ExternalInput").ap()
    idx_cat = nc.dram_tensor("idx_cat", [128, 2], I32, kind="ExternalInput").ap()
    d1t = nc.dram_tensor("d1t", [128, BL], BF16, kind="ExternalInput").ap()
    tops = nc.dram_tensor("tops", [128, 4, BL], BF16, kind="ExternalInput").ap()
    w1t = nc.dram_tensor("w1t", [128, KC1 * HIDDEN], BF16, kind="ExternalInput").ap()
    b1r = nc.dram_tensor("b1r", [128, KC2], F32, kind="ExternalInput").ap()
    b2r = nc.dram_tensor("b2r", [1, NOC * OCP], BF16, kind="ExternalInput").ap()
    ones_d = nc.dram_tensor("ones_d", [1, 128], BF16, kind="ExternalInput").ap()
    ident_d = nc.dram_tensor("ident_d", [128, 128], BF16, kind="ExternalInput").ap()
    w2t = nc.dram_tensor("w2t", [NOC, 128, KC2 * OCP], FP8,
                         kind="ExternalInput").ap()
    cat_e = nc.dram_tensor("cat_e", [CAT_V, SYN], BF16, kind="ExternalInput").ap()
    hve = nc.dram_tensor("hve", [HVEC_V + 1, SEM], BF16, kind="ExternalInput").ap()
    out_d = nc.dram_tensor("out", [BL, OUT], BF16, kind="ExternalOutput").ap()

    with tile.TileContext(nc) as tc:
        with __import__("contextlib").ExitStack() as ctx:
            cp = ctx.enter_context(tc.tile_pool(name="const", bufs=1))
            gp = ctx.enter_context(tc.tile_pool(name="gath", bufs=1))
            wp = ctx.enter_context(tc.tile_pool(name="work", bufs=1))
            accp = ctx.enter_context(tc.tile_pool(name="accp", bufs=3))
            ep = ctx.enter_context(tc.tile_pool(name="expp", bufs=2))
            ps_tr = ctx.enter_context(tc.tile_pool(name="ps_tr", bufs=2, space="PSUM"))
            ps_l1 = ctx.enter_context(tc.tile_pool(name="ps_l1", bufs=1, space="PSUM"))
            ps_l2 = ctx.enter_context(tc.tile_pool(name="ps_l2", bufs=2, space="PSUM"))

            # index loads first: the gathers are the phase-1 critical path
            ihv = cp.tile([128, 32], I32)
            nc.sync.dma_start(ihv[:], idx_hv)
            icat = cp.tile([128, 2], I32)
            nc.sync.dma_start(icat[:], idx_cat)
            # identity comes from DRAM: building it on GPSIMD would queue
            # behind the 34 gather dispatches and stall every transpose
            ident = cp.tile([128, 128], BF16)
            nc.sync.dma_start(ident[:], ident_d)

            # single-row indirect gathers ONLY: the HW SWDGE ucode does not
            # implement multi-offset APs (data lands permuted/unwritten).
            # Gathering into slices of one tile keeps the paired add tree.
            hvg = []
            for bh in range(2):
                g = gp.tile([128, 16, SEM], BF16, tag=f"hvg{bh}", name=f"hvg{bh}")
                hvg.append(g)
            cg = gp.tile([128, 2, SYN], BF16, tag="cg", name="cg")

            def gather(bh, t):
                for j in range(NNZ):
                    col = bh * 16 + t * NNZ + j
                    nc.gpsimd.indirect_dma_start(
                        out=hvg[bh][:, t * NNZ + j, :], out_offset=None,
                        in_=hve,
                        in_offset=IndirectOffsetOnAxis(
                            ap=ihv[:, col:col + 1], axis=0))

            def gather_cat(bh):
                nc.gpsimd.indirect_dma_start(
                    out=cg[:, bh, :], out_offset=None, in_=cat_e,
                    in_offset=IndirectOffsetOnAxis(ap=icat[:, bh:bh + 1],
                                                   axis=0))

            # bh0 gathers only — bh1's are emitted AFTER build0 so that
            # build0's DMA-semaphore waits do not cover the whole gather set
            gather_cat(0)
            gather(0, 0)
            gather(0, 1)

            xTb = []
            hTb = []
            for bh in range(2):
                xTb.append(cp.tile([128, KC1, 128], BF16, tag=f"xT{bh}",
                                   name=f"xT{bh}"))
                hTb.append(cp.tile([128, KC2, 128], FP8, tag=f"hT{bh}",
                                   name=f"hT{bh}"))
            for bh in range(2):
                nc.sync.dma_start(xTb[bh][:, 5, :],
                                  d1t[:, bh * 128:(bh + 1) * 128])
            b1t = cp.tile([128, KC2], F32)
            nc.sync.dma_start(b1t[:], b1r)
            topst = cp.tile([128, 4, BL], BF16)
            nc.sync.dma_start(topst[:], tops)
            w1tt = cp.tile([128, KC1, HIDDEN], BF16)
            nc.sync.dma_start(w1tt[:].rearrange("p a b -> p (a b)"), w1t)
            if has_b2:
                ones = cp.tile([1, 128], BF16)
                nc.sync.dma_start(ones[:], ones_d)
                b2sb = cp.tile([1, NOC, OCP], BF16)
                nc.sync.dma_start(b2sb[:].rearrange("p a b -> p (a b)"), b2r)

            # prefetch ALL W2 chunks: fp8 makes 20 x 4KB/partition fit in
            # SBUF; the DMA queue then streams continuously from t=0
            w2sb = []
            for oc in range(NOC):
                w = cp.tile([128, KC2, OCP], FP8, tag=f"w2_{oc}", name=f"w2_{oc}")
                nc.sync.dma_start(w[:].rearrange("p a b -> p (a b)"), w2t[oc])
                w2sb.append(w)

            y_sb = cp.tile([128, 2, OUT], BF16)
            sums = cp.tile([128, 2, NSC], F32)
            s1 = cp.tile([128, 2], F32)
            lgs = cp.tile([128, 2], F32)

            def bag_tree(bh, t):
                # 8 gathered rows -> 1 bag sum, 3 paired DVE adds
                g8 = hvg[bh][:, t * NNZ:(t + 1) * NNZ, :]
                gv = g8.rearrange("p (a two) c -> p a two c", two=2)
                s4 = wp.tile([128, 4, SEM], BF16, tag="s4", name=f"s4_{bh}_{t}")
                nc.vector.tensor_add(s4[:], gv[:, :, 0, :], gv[:, :, 1, :])
                s4v = s4[:].rearrange("p (a two) c -> p a two c", two=2)
                s2 = wp.tile([128, 2, SEM], BF16, tag="s2", name=f"s2_{bh}_{t}")
                nc.vector.tensor_add(s2[:], s4v[:, :, 0, :], s4v[:, :, 1, :])
                acc = accp.tile([128, SEM], BF16, tag="acc",
                                name=f"acc_{bh}_{t}")
                nc.vector.tensor_add(acc[:], s2[:, 0, :], s2[:, 1, :])
                return acc

            def build_x_and_l1(bh):
                # k-outer L1: partial-accumulate each contraction chunk as its
                # x slice arrives (d_onehot first, cat next, bag sums last)
                ph = [ps_l1.tile([128, 4, 128], F32, space="PSUM", tag=f"l1{i}",
                                 name=f"l1ph{bh}_{i}") for i in range(2)]

                def l1_level(ko, k):
                    for m in range(KC2):
                        nc.tensor.matmul(ph[m // 4][:, m % 4, :],
                                         w1tt[:, k, m * 128:(m + 1) * 128],
                                         xTb[bh][:, k, :], start=(ko == 0),
                                         stop=(ko == KC1 - 1))

                l1_level(0, 5)                     # d_onehot (direct DMA)
                pt = ps_tr.tile([128, 128], BF16, space="PSUM", tag="pt",
                                name=f"ptc{bh}")
                nc.tensor.transpose(out=pt[:], in_=cg[:, bh, :], identity=ident[:])
                nc.vector.tensor_copy(xTb[bh][:, 0, :], pt[:])
                l1_level(1, 0)                     # cat embed
                for t in range(2):
                    acc = bag_tree(bh, t)
                    for fh in range(2):
                        pt = ps_tr.tile([128, 128], BF16, space="PSUM", tag="pt",
                                        name=f"pt{bh}_{t}_{fh}")
                        nc.tensor.transpose(
                            out=pt[:], in_=acc[:, fh * 128:(fh + 1) * 128],
                            identity=ident[:])
                        nc.vector.tensor_add(
                            xTb[bh][:, 1 + t * 2 + fh, :],
                            pt[:], topst[:, t * 2 + fh, bh * 128:(bh + 1) * 128])
                        l1_level(2 + t * 2 + fh, 1 + t * 2 + fh)
                for m in range(KC2):
                    nc.vector.tensor_scalar(
                        out=hTb[bh][:, m, :], in0=ph[m // 4][:, m % 4, :],
                        scalar1=b1t[:, m:m + 1], scalar2=0.0,
                        op0=mybir.AluOpType.add, op1=mybir.AluOpType.max)

            def l2_group(so, bh, skip_copy=False):
                py = ps_l2.tile([128, 2, OCP], F32, space="PSUM", tag="l2",
                                name=f"py{so}_{bh}")
                # fp8 DoubleRow: contraction pairs (2q, 2q+1); the hT pair is
                # the stationary operand, reused across both 512-col chunks
                for q in range(KC2 // 2):
                    for r in range(2):
                        nc.tensor.matmul(
                            py[:, r, :], hTb[bh][:, 2 * q:2 * q + 2, :],
                            w2sb[2 * so + r][:, 2 * q:2 * q + 2, :],
                            start=(q == 0),
                            stop=(q == KC2 // 2 - 1 and not has_b2),
                            perf_mode=mybir.MatmulPerfMode.DoubleRow)
                if has_b2:
                    for r in range(2):
                        nc.tensor.matmul(py[:, r, :], ones[:],
                                         b2sb[:, 2 * so + r, :],
                                         start=False, stop=True)
                esc = ep.tile([128, 2, OCR], BF16, tag="esc", name=f"esc{so}_{bh}")
                nc.scalar.activation(
                    esc[:], py[:, :, :OCR],
                    mybir.ActivationFunctionType.Exp,
                    accum_out=sums[:, bh, so:so + 1])
                if not skip_copy:
                    yv = y_sb[:, bh, 2 * so * OCR:(2 * so + 2) * OCR].rearrange(
                        "p (a b) -> p a b", a=2)
                    # GPSIMD has no PSUM port: PSUM->SBUF copies must be DVE
                    nc.vector.tensor_copy(yv, py[:, :, :OCR])
                return py

            def finale_ln(bh):
                nc.vector.reduce_sum(s1[:, bh:bh + 1], sums[:, bh, :],
                                     axis=mybir.AxisListType.X)
                nc.scalar.activation(lgs[:, bh:bh + 1], s1[:, bh:bh + 1],
                                     mybir.ActivationFunctionType.Ln)

            # GPSIMD elementwise ops are ucode (slow + AP-scalar broken):
            # keep the finale entirely on DVE
            def finale_chunk(bh, q, src=None):
                qsl = slice(2 * q * OCR, (2 * q + 2) * OCR)
                out = y_sb[:, bh, qsl]
                if src is None:
                    in0 = out          # in-place
                else:   # subtract straight from PSUM: saves the copy in the tail
                    in0 = src
                    out = out.rearrange("p (a b) -> p a b", a=2)
                nc.vector.tensor_scalar(
                    out=out, in0=in0,
                    scalar1=lgs[:, bh:bh + 1], scalar2=None,
                    op0=mybir.AluOpType.subtract)
                # alternate output DMAs over two HWDGE queues so the ~0.6us
                # per-DMA dispatch does not serialize the tail
                eng = nc.sync if q % 2 == 0 else nc.scalar
                eng.dma_start(out_d[bh * 128:(bh + 1) * 128, qsl],
                              y_sb[:, bh, qsl])

            # ---- schedule: ALL bh0 groups first, so finale(0) (Ln, subtract,
            # output DMA) fully overlaps bh1's matmul stream; only finale(1)
            # remains as the serial tail ----
            build_x_and_l1(0)
            gather_cat(1)
            gather(1, 0)
            gather(1, 1)
            for so in range(NSC):
                l2_group(so, 0)
                if so == 8:
                    build_x_and_l1(1)
            l2_group(0, 1)
            finale_ln(0)
            for so in range(1, NSC - 1):
                l2_group(so, 1)
                finale_chunk(0, so - 1)
            py_last = l2_group(NSC - 1, 1, skip_copy=True)
            finale_chunk(0, 8)
            finale_chunk(0, 9)
            finale_ln(1)
            for q in range(NSC - 1):
                finale_chunk(1, q)
            finale_chunk(1, NSC - 1, src=py_last[:, :, :OCR])

    nc.compile()
    return nc


def _dedup_int32(ix):
    """Set semantics: within each row, later duplicates -> HVEC_V (zero row)."""
    ix = np.asarray(ix, dtype=np.int64)
    dup = ix[:, :, None] == ix[:, None, :]
    earlier = np.tril(np.ones((NNZ, NNZ), dtype=bool), -1)
    isdup = (dup & earlier[None]).any(axis=2)
    return np.where(isdup, HVEC_V, ix).astype(np.int32)


def _prep_inputs(d_onehot, cat_b_ix, hvb_ix, hvf_ix, hvb_top, hvf_top,
                 cat_embeds, hvec_embeds, W1, b1, W2, b2):
    d_onehot = np.asarray(d_onehot, np.float32)
    cat_b_ix = np.asarray(cat_b_ix).astype(np.int32)
    hv_clean = [_dedup_int32(hvb_ix), _dedup_int32(hvf_ix)]
    hv_top = [np.asarray(hvb_top, np.float32), np.asarray(hvf_top, np.float32)]
    # embedding tables scaled x32 (fp8 dynamic range); the x32 is divided
    # back out of the matching W1 columns below, so h is unchanged
    cat_embeds = np.ascontiguousarray(
        (np.asarray(cat_embeds, np.float32) * EMB_S).astype(BF16NP))
    hve_aug = np.concatenate(
        [np.asarray(hvec_embeds, np.float32) * EMB_S,
         np.zeros((1, SEM), np.float32)], axis=0)
    hve_aug = np.ascontiguousarray(hve_aug.astype(BF16NP))

    W1a = np.asarray(W1, np.float32) / W2_S
    W1a = W1a.copy()
    W1a[:, :IN_DIM - 7] /= EMB_S       # undo the x32 on cat+hv blocks
    w1t_pad = np.zeros((KC1 * 128, HIDDEN), np.float32)
    w1t_pad[:IN_DIM] = W1a.T
    b1r = np.ascontiguousarray(
        (np.asarray(b1, np.float32) / W2_S).reshape(KC2, 128).T)

    # W2 x4 (h carries the 1/4): keeps fp8 e4m3 out of the subnormal range
    W2q = (np.asarray(W2, np.float32) * W2_S).astype(FP8NP)
    W2q = W2q.reshape(NOC, OCR, KC2, 128).transpose(0, 3, 2, 1)  # oc,p,kc,c
    w2pad = np.zeros((NOC, 128, KC2, OCP), FP8NP)
    w2pad[:, :, :, :OCR] = W2q
    w2t_f8 = np.ascontiguousarray(w2pad.reshape(NOC, 128, KC2 * OCP))

    w1t_bf = np.ascontiguousarray(
        w1t_pad.astype(BF16NP).reshape(KC1, 128, HIDDEN)
        .transpose(1, 0, 2).reshape(128, KC1 * HIDDEN))
    b2pad = np.zeros((NOC, OCP), np.float32)
    b2pad[:, :OCR] = np.asarray(b2, np.float32).reshape(NOC, OCR)
    b2r_bf = np.ascontiguousarray(b2pad.reshape(1, NOC * OCP).astype(BF16NP))

    in_maps = []
    for c in range(N_CORES):
        rs = slice(c * BL, (c + 1) * BL)
        ihv = np.zeros((128, 32), np.int32)
        icat = np.zeros((128, 2), np.int32)
        d1t = np.zeros((128, BL), BF16NP)
        topst = np.zeros((128, 4, BL), np.float32)
        for bh in range(2):
            brs = slice(c * BL + bh * 128, c * BL + (bh + 1) * 128)
            icat[:, bh] = cat_b_ix[brs]
            for t in range(2):
                base = bh * 16 + t * NNZ
                ihv[:, base:base + NNZ] = hv_clean[t][brs]
        d1t[:7, :] = d_onehot[rs].T
        for t in range(2):
            for fh in range(2):
                topst[:, t * 2 + fh, :] = \
                    hv_top[t][rs, fh * 128:(fh + 1) * 128].T * EMB_S
        in_maps.append({
            "idx_hv": ihv, "idx_cat": icat, "d1t": d1t,
            "tops": np.ascontiguousarray(topst.astype(BF16NP)),
            "w1t": w1t_bf, "b1r": b1r,
            "b2r": b2r_bf, "w2t": w2t_f8, "cat_e": cat_embeds, "hve": hve_aug,
            "ones_d": np.ones((1, 128), BF16NP),
            "ident_d": np.eye(128, dtype=np.float32).astype(BF16NP),
        })
    return in_maps


def _ensure_ntff_hook():
    try:
        from antenv.axon_hooks import get_axon_ntff_profile_hook  # noqa: F401
        return True
    except ImportError:
        pass
    try:
        import antenv
        mod = types.ModuleType("antenv.axon_hooks")
        _h = {}
        mod.set_axon_ntff_profile_hook = lambda h: _h.__setitem__("h", h)
        mod.get_axon_ntff_profile_hook = lambda: _h.get("h")
        sys.modules["antenv.axon_hooks"] = mod
        antenv.axon_hooks = mod
        from trn_agent_boot.trn_boot import _ntff_profile_via_ctypes
        h = _ntff_profile_via_ctypes("/opt/axon/libaxon_pjrt.so")
        if h is not None:
            mod.set_axon_ntff_profile_hook(h)
            return True
    except Exception:
        pass
    return False


def _run(inputs, trace=False):
    has_b2 = bool(np.any(np.asarray(inputs["b2"], np.float32)))
    key = ("nc", has_b2)
    if key not in _STATE:
        _STATE[key] = _build_program(has_b2)
    nc = _STATE[key]
    in_maps = _prep_inputs(**inputs)
    if trace:
        _ensure_ntff_hook()
    last_err = None
    for _attempt in range(2):
        try:
            res = run_bass_kernel_spmd(nc, in_maps,
                                       core_ids=list(range(N_CORES)),
                                       trace=trace)
            break
        except Exception as e:  # flaky first-exec device fault; retry
            last_err = e
            import time as _time
            _time.sleep(2.0)
    else:
        raise last_err
    out = np.concatenate(
        [res.results[c]["out"].astype(np.float32) for c in range(N_CORES)],
        axis=0)
    return out, res


def kernel(**inputs):
    try:
        out, _ = _run(inputs, trace=False)
        return out
    except Exception:
        pass
    # Fresh-session retries: the first execution of a newly compiled NEFF
    # occasionally faults the device; a new process/session recovers.
    import os
    import pickle
    import subprocess
    import tempfile
    import time
    last = None
    for attempt in range(4):
        time.sleep(2.0 * (attempt + 1))
        td = tempfile.mkdtemp()
        inp = os.path.join(td, "in.pkl")
        outp = os.path.join(td, "out.npy")
        with open(inp, "wb") as f:
            pickle.dump(inputs, f)
        try:
            r = subprocess.run([sys.executable, os.path.abspath(__file__),
                                "--subproc", inp, outp], timeout=1200)
            if r.returncode == 0 and os.path.exists(outp):
                return np.load(outp)
        except Exception as e:
            last = e
    raise RuntimeError(f"kernel failed after retries: {last}")


def _subproc_main(inp, outp):
    with open(inp, "rb") as f:
        inputs = pickle.load(f)
    out, _ = _run(inputs, trace=False)
    np.save(outp, out)


if __name__ == "__main__" and len(sys.argv) >= 4 and sys.argv[1] == "--subproc":
    import pickle
    _subproc_main(sys.argv[2], sys.argv[3])


# revision 31
# speedup vs baseline: 1.0293x; 1.0293x over previous
"""Trainium2 Bass kernel for an embedding-bag + 2-layer MLP + log_softmax model.

Model (reference semantics):
  cat = cat_embeds[cat_b_ix]                       # (B, 128)
  hvb = multihot(hvb_ix) @ hvec_embeds + hvb_top   # (B, 256)  (set semantics)
  hvf = multihot(hvf_ix) @ hvec_embeds + hvf_top   # (B, 256)
  x   = [cat | hvb | hvf | d_onehot]               # (B, 647)
  h   = relu(x @ W1.T + b1)                        # (B, 1024)
  y   = h @ W2.T + b2                              # (B, 10000)
  out = log_softmax(y, axis=1)

Sharding: data-parallel over 8 NeuronCores (256 batch rows each);
embedding tables and weights replicated.

Device kernel (per core), v2:
  - batched indirect-DMA row gathers (one instruction per 8-row bag pair;
    SWDGE cost is ~1us fixed per instruction, so batching cuts gather
    dispatch from ~37us to ~6us)
  - hvec table stored fp8e4 scaled x32 (halves gather bytes); scale folded
    into W1 columns on the host, so no on-device rescale
  - L1 matmul bf16 -> h stored fp8e4 (W2 scaled x4 on host, W1/b1 scaled
    1/4, so h is h/4: keeps W2 out of the fp8 subnormal range)
  - L2 matmul in fp8 DoubleRow perf mode: 2 fp8 weights per PE cell,
    halves both the W2 HBM traffic (10.5MB vs 20.5MB) and the PE stream
    time; W2 chunks padded 500->512 cols so the k-pair stride is 16B
    aligned
  - all 20 W2 chunks prefetched up front (fp8 makes them fit in SBUF),
    DMA streams continuously
  - fused exp+row-sum on ACT (logits are small, no max-shift needed),
    PSUM->SBUF y copies alternate DVE/GPSIMD, log-softmax subtraction
    split DVE/GPSIMD, bf16 output (upcast on host)
"""

import sys
import types

import ml_dtypes
import numpy as np

BF16NP = ml_dtypes.bfloat16
FP8NP = ml_dtypes.float8_e4m3

import concourse.bacc as bacc
import concourse.bass as bass
import concourse.mybir as mybir
import concourse.tile as tile
from concourse.bass import IndirectOffsetOnAxis
from concourse.bass_utils import run_bass_kernel_spmd

F32 = mybir.dt.float32
BF16 = mybir.dt.bfloat16
FP8 = mybir.dt.float8e4
I32 = mybir.dt.int32

N_CORES = 8
B = 2048
BL = B // N_CORES          # 256 batch rows per core
NNZ = 8
CAT_V = 10000
HVEC_V = 50000
SYN = 128
SEM = 256
HIDDEN = 1024
OUT = 10000
IN_DIM = 7 + SYN + 2 * SEM     # 647
KC1 = 6                        # L1 contraction chunks (768 = 6*128, padded)
KC2 = HIDDEN // 128            # 8
OCR = 500                      # real output cols per chunk (10000 = 20*500)
OCP = 512                      # padded chunk width (16B-aligned kc stride)
NOC = OUT // OCR               # 20
NSC = NOC // 2                 # 10 super-chunks of 2 chunks each
EMB_S = 1.0                    # embedding tables kept bf16: no rescale
W2_S = 4.0                     # W2 scaled x4, h scaled 1/4

_STATE = {}


def _build_program(has_b2):
    nc = bacc.Bacc("TRN2", target_bir_lowering=False, debug=False,
                   num_devices=N_CORES, dynamic_dma_scratch_size=16384)

    idx_hv = nc.dram_tensor("idx_hv", [128, 32], I32, kind="ExternalInput").ap()
    idx_cat = nc.dram_tensor("idx_cat", [128, 2], I32, kind="ExternalInput").ap()
    d1t = nc.dram_tensor("d1t", [128, BL], BF16, kind="ExternalInput").ap()
    tops = nc.dram_tensor("tops", [128, 4, BL], BF16, kind="ExternalInput").ap()
    w1t = nc.dram_tensor("w1t", [128, KC1 * HIDDEN], BF16, kind="ExternalInput").ap()
    b1r = nc.dram_tensor("b1r", [128, KC2], F32, kind="ExternalInput").ap()
    b2r = nc.dram_tensor("b2r", [1, NOC * OCP], BF16, kind="ExternalInput").ap()
    ones_d = nc.dram_tensor("ones_d", [1, 128], BF16, kind="ExternalInput").ap()
    ident_d = nc.dram_tensor("ident_d", [128, 128], BF16, kind="ExternalInput").ap()
    w2t = nc.dram_tensor("w2t", [NOC, 128, KC2 * OCP], FP8,
                         kind="ExternalInput").ap()
    cat_e = nc.dram_tensor("cat_e", [CAT_V, SYN], BF16, kind="ExternalInput").ap()
    hve = nc.dram_tensor("hve", [HVEC_V + 1, SEM], BF16, kind="ExternalInput").ap()
    out_d = nc.dram_tensor("out", [BL, OUT], BF16, kind="ExternalOutput").ap()

    with tile.TileContext(nc) as tc:
        with __import__("contextlib").ExitStack() as ctx:
            cp = ctx.enter_context(tc.tile_pool(name="const", bufs=1))
            gp = ctx.enter_context(tc.tile_pool(name="gath", bufs=1))
            wp = ctx.enter_context(tc.tile_pool(name="work", bufs=1))
            accp = ctx.enter_context(tc.tile_pool(name="accp", bufs=3))
            ep = ctx.enter_context(tc.tile_pool(name="expp", bufs=2))
            ps_tr = ctx.enter_context(tc.tile_pool(name="ps_tr", bufs=2, space="PSUM"))
            ps_l1 = ctx.enter_context(tc.tile_pool(name="ps_l1", bufs=1, space="PSUM"))
            ps_l2 = ctx.enter_context(tc.tile_pool(name="ps_l2", bufs=2, space="PSUM"))

            # index loads first: the gathers are the phase-1 critical path
            ihv = cp.tile([128, 32], I32)
            nc.sync.dma_start(ihv[:], idx_hv)
            icat = cp.tile([128, 2], I32)
            nc.sync.dma_start(icat[:], idx_cat)
            # identity comes from DRAM: building it on GPSIMD would queue
            # behind the 34 gather dispatches and stall every transpose
            ident = cp.tile([128, 128], BF16)
            nc.sync.dma_start(ident[:], ident_d)

            # single-row indirect gathers ONLY: the HW SWDGE ucode does not
            # implement multi-offset APs (data lands permuted/unwritten).
            # Gathering into slices of one tile keeps the paired add tree.
            hvg = []
            for bh in range(2):
                g = gp.tile([128, 16, SEM], BF16, tag=f"hvg{bh}", name=f"hvg{bh}")
                hvg.append(g)
            cg = gp.tile([128, 2, SYN], BF16, tag="cg", name="cg")

            def gather(bh, t):
                for j in range(NNZ):
                    col = bh * 16 + t * NNZ + j
                    nc.gpsimd.indirect_dma_start(
                        out=hvg[bh][:, t * NNZ + j, :], out_offset=None,
                        in_=hve,
                        in_offset=IndirectOffsetOnAxis(
                            ap=ihv[:, col:col + 1], axis=0))

            def gather_cat(bh):
                nc.gpsimd.indirect_dma_start(
                    out=cg[:, bh, :], out_offset=None, in_=cat_e,
                    in_offset=IndirectOffsetOnAxis(ap=icat[:, bh:bh + 1],
                                                   axis=0))

            # bh0 gathers only — bh1's are emitted AFTER build0 so that
            # build0's DMA-semaphore waits do not cover the whole gather set
            gather_cat(0)
            gather(0, 0)
            gather(0, 1)

            xTb = []
            hTb = []
            for bh in range(2):
                xTb.append(cp.tile([128, KC1, 128], BF16, tag=f"xT{bh}",
                                   name=f"xT{bh}"))
                hTb.append(cp.tile([128, KC2, 128], FP8, tag=f"hT{bh}",
                                   name=f"hT{bh}"))
            for bh in range(2):
                nc.sync.dma_start(xTb[bh][:, 5, :],
                                  d1t[:, bh * 128:(bh + 1) * 128])
            b1t = cp.tile([128, KC2], F32)
            nc.sync.dma_start(b1t[:], b1r)
            topst = cp.tile([128, 4, BL], BF16)
            nc.sync.dma_start(topst[:], tops)
            w1tt = cp.tile([128, KC1, HIDDEN], BF16)
            nc.sync.dma_start(w1tt[:].rearrange("p a b -> p (a b)"), w1t)
            if has_b2:
                ones = cp.tile([1, 128], BF16)
                nc.sync.dma_start(ones[:], ones_d)
                b2sb = cp.tile([1, NOC, OCP], BF16)
                nc.sync.dma_start(b2sb[:].rearrange("p a b -> p (a b)"), b2r)

            # prefetch ALL W2 chunks: fp8 makes 20 x 4KB/partition fit in
            # SBUF; the DMA queue then streams continuously from t=0
            w2sb = []
            for oc in range(NOC):
                w = cp.tile([128, KC2, OCP], FP8, tag=f"w2_{oc}", name=f"w2_{oc}")
                nc.sync.dma_start(w[:].rearrange("p a b -> p (a b)"), w2t[oc])
                w2sb.append(w)

            y_sb = cp.tile([128, 2, OUT], BF16)
            sums = cp.tile([128, 2, NSC], F32)
            s1 = cp.tile([128, 2], F32)
            lgs = cp.tile([128, 2], F32)

            def bag_tree(bh, t):
                # 8 gathered rows -> 1 bag sum, 3 paired DVE adds
                g8 = hvg[bh][:, t * NNZ:(t + 1) * NNZ, :]
                gv = g8.rearrange("p (a two) c -> p a two c", two=2)
                s4 = wp.tile([128, 4, SEM], BF16, tag="s4", name=f"s4_{bh}_{t}")
                nc.vector.tensor_add(s4[:], gv[:, :, 0, :], gv[:, :, 1, :])
                s4v = s4[:].rearrange("p (a two) c -> p a two c", two=2)
                s2 = wp.tile([128, 2, SEM], BF16, tag="s2", name=f"s2_{bh}_{t}")
                nc.vector.tensor_add(s2[:], s4v[:, :, 0, :], s4v[:, :, 1, :])
                acc = accp.tile([128, SEM], BF16, tag="acc",
                                name=f"acc_{bh}_{t}")
                nc.vector.tensor_add(acc[:], s2[:, 0, :], s2[:, 1, :])
                return acc

            def build_x_and_l1(bh):
                # k-outer L1: partial-accumulate each contraction chunk as its
                # x slice arrives (d_onehot first, cat next, bag sums last)
                ph = [ps_l1.tile([128, 4, 128], F32, space="PSUM", tag=f"l1{i}",
                                 name=f"l1ph{bh}_{i}") for i in range(2)]

                def l1_level(ko, k):
                    for m in range(KC2):
                        nc.tensor.matmul(ph[m // 4][:, m % 4, :],
                                         w1tt[:, k, m * 128:(m + 1) * 128],
                                         xTb[bh][:, k, :], start=(ko == 0),
                                         stop=(ko == KC1 - 1))

                l1_level(0, 5)                     # d_onehot (direct DMA)
                pt = ps_tr.tile([128, 128], BF16, space="PSUM", tag="pt",
                                name=f"ptc{bh}")
                nc.tensor.transpose(out=pt[:], in_=cg[:, bh, :], identity=ident[:])
                nc.vector.tensor_copy(xTb[bh][:, 0, :], pt[:])
                l1_level(1, 0)                     # cat embed
                for t in range(2):
                    acc = bag_tree(bh, t)
                    for fh in range(2):
                        pt = ps_tr.tile([128, 128], BF16, space="PSUM", tag="pt",
                                        name=f"pt{bh}_{t}_{fh}")
                        nc.tensor.transpose(
                            out=pt[:], in_=acc[:, fh * 128:(fh + 1) * 128],
                            identity=ident[:])
                        nc.vector.tensor_add(
                            xTb[bh][:, 1 + t * 2 + fh, :],
                            pt[:], topst[:, t * 2 + fh, bh * 128:(bh + 1) * 128])
                        l1_level(2 + t * 2 + fh, 1 + t * 2 + fh)
                for m in range(KC2):
                    nc.vector.tensor_scalar(
                        out=hTb[bh][:, m, :], in0=ph[m // 4][:, m % 4, :],
                        scalar1=b1t[:, m:m + 1], scalar2=0.0,
                        op0=mybir.AluOpType.add, op1=mybir.AluOpType.max)

            def l2_group(so, bh, skip_copy=False):
                py = ps_l2.tile([128, 2, OCP], F32, space="PSUM", tag="l2",
                                name=f"py{so}_{bh}")
                # fp8 DoubleRow: contraction pairs (2q, 2q+1); the hT pair is
                # the stationary operand, reused across both 512-col chunks
                for q in range(KC2 // 2):
                    for r in range(2):
                        nc.tensor.matmul(
                            py[:, r, :], hTb[bh][:, 2 * q:2 * q + 2, :],
                            w2sb[2 * so + r][:, 2 * q:2 * q + 2, :],
                            start=(q == 0),
                            stop=(q == KC2 // 2 - 1 and not has_b2),
                            perf_mode=mybir.MatmulPerfMode.DoubleRow)
                if has_b2:
                    for r in range(2):
                        nc.tensor.matmul(py[:, r, :], ones[:],
                                         b2sb[:, 2 * so + r, :],
                                         start=False, stop=True)
                esc = ep.tile([128, 2, OCR], BF16, tag="esc", name=f"esc{so}_{bh}")
                nc.scalar.activation(
                    esc[:], py[:, :, :OCR],
                    mybir.ActivationFunctionType.Exp,
                    accum_out=sums[:, bh, so:so + 1])
                if not skip_copy:
                    yv = y_sb[:, bh, 2 * so * OCR:(2 * so + 2) * OCR].rearrange(
                        "p (a b) -> p a b", a=2)
                    # GPSIMD has no PSUM port: PSUM->SBUF copies must be DVE
                    nc.vector.tensor_copy(yv, py[:, :, :OCR])
                return py

            def finale_ln(bh):
                nc.vector.reduce_sum(s1[:, bh:bh + 1], sums[:, bh, :],
                                     axis=mybir.AxisListType.X)
                nc.scalar.activation(lgs[:, bh:bh + 1], s1[:, bh:bh + 1],
                                     mybir.ActivationFunctionType.Ln)

            # GPSIMD elementwise ops are ucode (slow + AP-scalar broken):
            # keep the finale entirely on DVE
            def finale_chunk(bh, q, src=None):
                qsl = slice(2 * q * OCR, (2 * q + 2) * OCR)
                out = y_sb[:, bh, qsl]
                if src is None:
                    in0 = out          # in-place
                else:   # subtract straight from PSUM: saves the copy in the tail
                    in0 = src
                    out = out.rearrange("p (a b) -> p a b", a=2)
                nc.vector.tensor_scalar(
                    out=out, in0=in0,
                    scalar1=lgs[:, bh:bh + 1], scalar2=None,
                    op0=mybir.AluOpType.subtract)
                # alternate output DMAs over two HWDGE queues so the ~0.6us
                # per-DMA dispatch does not serialize the tail
                eng = nc.sync if q % 2 == 0 else nc.scalar
                eng.dma_start(out_d[bh * 128:(bh + 1) * 128, qsl],
                              y_sb[:, bh, qsl])

            # ---- schedule: ALL bh0 groups first, so finale(0) (Ln, subtract,
            # output DMA) fully overlaps bh1's matmul stream; only finale(1)
            # remains as the serial tail ----
            build_x_and_l1(0)
            gather_cat(1)
            gather(1, 0)
            gather(1, 1)
            for so in range(NSC):
                l2_group(so, 0)
                if so == 7:
                    build_x_and_l1(1)
            l2_group(0, 1)
            finale_ln(0)
            for so in range(1, NSC - 1):
                l2_group(so, 1)
                q0 = (so - 1) * 2
                for q in (q0, q0 + 1):
                    if q < NSC:
                        finale_chunk(0, q)
            py_last = l2_group(NSC - 1, 1, skip_copy=True)
            finale_ln(1)
            for q in range(NSC - 1):
                finale_chunk(1, q)
            finale_chunk(1, NSC - 1, src=py_last[:, :, :OCR])

    nc.compile()
    return nc


def _dedup_int32(ix):
    """Set semantics: within each row, later duplicates -> HVEC_V (zero row)."""
    ix = np.asarray(ix, dtype=np.int64)
    dup = ix[:, :, None] == ix[:, None, :]
    earlier = np.tril(np.ones((NNZ, NNZ), dtype=bool), -1)
    isdup = (dup & earlier[None]).any(axis=2)
    return np.where(isdup, HVEC_V, ix).astype(np.int32)


def _prep_inputs(d_onehot, cat_b_ix, hvb_ix, hvf_ix, hvb_top, hvf_top,
                 cat_embeds, hvec_embeds, W1, b1, W2, b2):
    d_onehot = np.asarray(d_onehot, np.float32)
    cat_b_ix = np.asarray(cat_b_ix).astype(np.int32)
    hv_clean = [_dedup_int32(hvb_ix), _dedup_int32(hvf_ix)]
    hv_top = [np.asarray(hvb_top, np.float32), np.asarray(hvf_top, np.float32)]
    # embedding tables scaled x32 (fp8 dynamic range); the x32 is divided
    # back out of the matching W1 columns below, so h is unchanged
    cat_embeds = np.ascontiguousarray(
        (np.asarray(cat_embeds, np.float32) * EMB_S).astype(BF16NP))
    hve_aug = np.concatenate(
        [np.asarray(hvec_embeds, np.float32) * EMB_S,
         np.zeros((1, SEM), np.float32)], axis=0)
    hve_aug = np.ascontiguousarray(hve_aug.astype(BF16NP))

    W1a = np.asarray(W1, np.float32) / W2_S
    W1a = W1a.copy()
    W1a[:, :IN_DIM - 7] /= EMB_S       # undo the x32 on cat+hv blocks
    w1t_pad = np.zeros((KC1 * 128, HIDDEN), np.float32)
    w1t_pad[:IN_DIM] = W1a.T
    b1r = np.ascontiguousarray(
        (np.asarray(b1, np.float32) / W2_S).reshape(KC2, 128).T)

    # W2 x4 (h carries the 1/4): keeps fp8 e4m3 out of the subnormal range
    W2q = (np.asarray(W2, np.float32) * W2_S).astype(FP8NP)
    W2q = W2q.reshape(NOC, OCR, KC2, 128).transpose(0, 3, 2, 1)  # oc,p,kc,c
    w2pad = np.zeros((NOC, 128, KC2, OCP), FP8NP)
    w2pad[:, :, :, :OCR] = W2q
    w2t_f8 = np.ascontiguousarray(w2pad.reshape(NOC, 128, KC2 * OCP))

    w1t_bf = np.ascontiguousarray(
        w1t_pad.astype(BF16NP).reshape(KC1, 128, HIDDEN)
        .transpose(1, 0, 2).reshape(128, KC1 * HIDDEN))
    b2pad = np.zeros((NOC, OCP), np.float32)
    b2pad[:, :OCR] = np.asarray(b2, np.float32).reshape(NOC, OCR)
    b2r_bf = np.ascontiguousarray(b2pad.reshape(1, NOC * OCP).astype(BF16NP))

    in_maps = []
    for c in range(N_CORES):
        rs = slice(c * BL, (c + 1) * BL)
        ihv = np.zeros((128, 32), np.int32)
        icat = np.zeros((128, 2), np.int32)
        d1t = np.zeros((128, BL), BF16NP)
        topst = np.zeros((128, 4, BL), np.float32)
        for bh in range(2):
            brs = slice(c * BL + bh * 128, c * BL + (bh + 1) * 128)
            icat[:, bh] = cat_b_ix[brs]
            for t in range(2):
                base = bh * 16 + t * NNZ
                ihv[:, base:base + NNZ] = hv_clean[t][brs]
        d1t[:7, :] = d_onehot[rs].T
        for t in range(2):
            for fh in range(2):
                topst[:, t * 2 + fh, :] = \
                    hv_top[t][rs, fh * 128:(fh + 1) * 128].T * EMB_S
        in_maps.append({
            "idx_hv": ihv, "idx_cat": icat, "d1t": d1t,
            "tops": np.ascontiguousarray(topst.astype(BF16NP)),
            "w1t": w1t_bf, "b1r": b1r,
            "b2r": b2r_bf, "w2t": w2t_f8, "cat_e": cat_embeds, "hve": hve_aug,
            "ones_d": np.ones((1, 128), BF16NP),
            "ident_d": np.eye(128, dtype=np.float32).astype(BF16NP),
        })
    return in_maps


def _ensure_ntff_hook():
    try:
        from antenv.axon_hooks import get_axon_ntff_profile_hook  # noqa: F401
        return True
    except ImportError:
        pass
    try:
        import antenv
        mod = types.ModuleType("antenv.axon_hooks")
        _h = {}
        mod.set_axon_ntff_profile_hook = lambda h: _h.__setitem__("h", h)
        mod.get_axon_ntff_profile_hook = lambda: _h.get("h")
        sys.modules["antenv.axon_hooks"] = mod
        antenv.axon_hooks = mod
        from trn_agent_boot.trn_boot import _ntff_profile_via_ctypes
        h = _ntff_profile_via_ctypes("/opt/axon/libaxon_pjrt.so")
        if h is not None:
            mod.set_axon_ntff_profile_hook(h)
            return True
    except Exception:
        pass
    return False


def _run(inputs, trace=False):
    has_b2 = bool(np.any(np.asarray(inputs["b2"], np.float32)))
    key = ("nc", has_b2)
    if key not in _STATE:
        _STATE[key] = _build_program(has_b2)
    nc = _STATE[key]
    in_maps = _prep_inputs(**inputs)
    if trace:
        _ensure_ntff_hook()
    last_err = None
    for _attempt in range(2):
        try:
            res = run_bass_kernel_spmd(nc, in_maps,
                                       core_ids=list(range(N_CORES)),
                                       trace=trace)
            break
        except Exception as e:  # flaky first-exec device fault; retry
            last_err = e
            import time as _time
            _time.sleep(2.0)
    else:
        raise last_err
    out = np.concatenate(
        [res.results[c]["out"].astype(np.float32) for c in range(N_CORES)],
        axis=0)
    return out, res


def kernel(**inputs):
    try:
        out, _ = _run(inputs, trace=False)
        return out
    except Exception:
        pass
    # Fresh-session retries: the first execution of a newly compiled NEFF
    # occasionally faults the device; a new process/session recovers.
    import os
    import pickle
    import subprocess
    import tempfile
    import time
    last = None
    for attempt in range(4):
        time.sleep(2.0 * (attempt + 1))
        td = tempfile.mkdtemp()
        inp = os.path.join(td, "in.pkl")
        outp = os.path.join(td, "out.npy")
        with open(inp, "wb") as f:
            pickle.dump(inputs, f)
        try:
            r = subprocess.run([sys.executable, os.path.abspath(__file__),
                                "--subproc", inp, outp], timeout=1200)
            if r.returncode == 0 and os.path.exists(outp):
                return np.load(outp)
        except Exception as e:
            last = e
    raise RuntimeError(f"kernel failed after retries: {last}")


def _subproc_main(inp, outp):
    with open(inp, "rb") as f:
        inputs = pickle.load(f)
    out, _ = _run(inputs, trace=False)
    np.save(outp, out)


if __name__ == "__main__" and len(sys.argv) >= 4 and sys.argv[1] == "--subproc":
    import pickle
    _subproc_main(sys.argv[2], sys.argv[3])
